# revision 63
# baseline (speedup 1.0000x reference)
"""Causal self-attention (B=2, T=2048, L=1024, H=16) on 8 TRN2 NeuronCores.

Sharding: tensor-parallel over heads (4 heads/core) x data-parallel over batch
(cores 0-3 -> batch 0, cores 4-7 -> batch 1). Each core computes its heads'
attention plus the partial output projection; the host sums the 4 f32
partials per batch.

v4 layout (HW-measured: per-matmul overhead dominates, so the design
minimizes streamed PSUM columns and instruction count):
  - QK projection in fp8e4m3 DoubleRow (256-contraction per matmul, halves
    column passes); V projection bf16 (fp8 V fails the 2e-2 gate).
  - Scores: plain fp8, K=64, per-(pr,hh) with heads at partition bases
    {0,64}; one matmul per (kc, head), N<=512.
  - PV hybrid: q-block 0 (queries 0-511) in bf16 (small softmax
    denominators amplify fp8 noise there); full blocks of q-blocks 1-3 via
    fp8 DoubleRow over kc pairs (256-key contraction per pass, M=66 with
    a 68-col padded head stride for 4B-aligned fp8 weight APs); diagonal
    blocks bf16. ones-column rides in va -> free softmax denominator.
  - phase1 (per nb) interleaved with phase2 (per qb); all x DMAs up front.
  - exp on Act (fp8 out for DR blocks); causal mask via gpsimd
    affine_select (Pool); normalize: po->SBUF on DVE, reciprocal DVE,
    denominator broadcast via DRAM round-trip DMA (keeps PE out of the
    normalize chain), yt muls in DVE fast mode.
  - PSUM note: start=True zeroes the whole 2KB bank, so only the first
    matmul touching a bank may set it.
"""

import sys

for _p in ("/opt/trn_rl_repo",):
    if _p not in sys.path:
        sys.path.insert(0, _p)

import numpy as np

import concourse.bass as bass
import concourse.mybir as mybir
import concourse.tile as tile

F32 = mybir.dt.float32
BF16 = mybir.dt.bfloat16
FP8 = mybir.dt.float8e4
EXP = mybir.ActivationFunctionType.Exp
DR = mybir.MatmulPerfMode.DoubleRow
import os as _os0
SCORES_PM = (mybir.MatmulPerfMode.DoublePixel
             if _os0.environ.get("SCORES_DP") else None)
NO_PROJ = bool(_os0.environ.get("KERNEL_NO_PROJ"))
NO_EXP = bool(_os0.environ.get("KERNEL_NO_EXP"))
NO_SCOREPV = bool(_os0.environ.get("KERNEL_NO_SCOREPV"))

B, T, L = 2, 2048, 1024
H = 16
DH = 64                      # head dim
HPC = 4                      # heads per core
HG = HPC * DH                # 256 cols per core per q/k/v
N_CORES = 8
NQB = T // 512               # 4 q-blocks
SCALE = 1.0 / np.sqrt(np.float32(L))  # rsqrt(L) per reference


def build_nc(reps=1):
    nc = bass.Bass("TRN2", target_bir_lowering=False, debug=False)

    xT8 = nc.dram_tensor("xT8", [L, T], FP8, kind="ExternalInput").ap()
    xT16 = nc.dram_tensor("xT16", [L, T], BF16, kind="ExternalInput").ap()
    wa8 = nc.dram_tensor("wa8", [L, 2 * HG], FP8, kind="ExternalInput").ap()
    wav = nc.dram_tensor("wav", [L, HG], BF16, kind="ExternalInput").ap()
    wp = nc.dram_tensor("wp", [HG, L], BF16, kind="ExternalInput").ap()
    msk = nc.dram_tensor("msk", [128, 128], BF16, kind="ExternalInput").ap()
    F16 = mybir.dt.float16
    out = nc.dram_tensor("out", [T, L], F16, kind="ExternalOutput").ap()
    scr = nc.dram_tensor("scr", [2, 1024], BF16, kind="Internal").ap()

    from contextlib import ExitStack

    with tile.TileContext(nc) as tc:
        with ExitStack() as stack:
            consts = stack.enter_context(tc.tile_pool(name="consts", bufs=1))
            xp8 = stack.enter_context(tc.tile_pool(name="xp8", bufs=12))
            xp16 = stack.enter_context(tc.tile_pool(name="xp16", bufs=8))
            wap = stack.enter_context(tc.tile_pool(name="wap", bufs=12))
            qkp = stack.enter_context(tc.tile_pool(name="qk", bufs=4))
            vp = stack.enter_context(tc.tile_pool(name="vp", bufs=1))
            ptp = stack.enter_context(tc.tile_pool(name="ptp", bufs=3))
            ptp8 = stack.enter_context(tc.tile_pool(name="ptp8", bufs=3))
            vp8 = stack.enter_context(tc.tile_pool(name="vp8", bufs=1))
            ytp = stack.enter_context(tc.tile_pool(name="ytp", bufs=3))
            recp = stack.enter_context(tc.tile_pool(name="recp", bufs=2))
            posb = stack.enter_context(tc.tile_pool(name="posb", bufs=4))
            bcp = stack.enter_context(tc.tile_pool(name="bcp", bufs=2))
            outp = stack.enter_context(tc.tile_pool(name="outp", bufs=3))
            # ---- constants & weights (resident across reps) ----
            msk_sb = consts.tile([128, 128], BF16)
            nc.gpsimd.dma_start(out=msk_sb[:], in_=msk[:])
            on1 = consts.tile([1, 64], BF16)
            nc.vector.memset(on1[:], 1.0)

            wa8_sb = []
            for kp in range(4):
                w = wap.tile([128, 2, 512], FP8, tag="wa8")
                nc.sync.dma_start(
                    out=w[:],
                    in_=wa8[kp * 256:(kp + 1) * 256, :].rearrange(
                        "(two p) c -> p two c", two=2),
                )
                wa8_sb.append(w)
            wav_sb = []
            for kc in range(8):
                w = wap.tile([128, HG], BF16, tag="wav")
                nc.sync.dma_start(out=w[:], in_=wav[kc * 128:(kc + 1) * 128, :])
                wav_sb.append(w)
            wp_sb = []
            for i in range(2):
                w = wap.tile([128, L], BF16, tag="wp")
                nc.gpsimd.dma_start(out=w[:], in_=wp[i * 128:(i + 1) * 128, :])
                wp_sb.append(w)

            # va tiles persist across reps; ones column written once.
            # bf16 singles (diag + qb0 blocks) and fp8 kc-pairs (DR full
            # blocks, queries >= 512).
            va_sb = []
            for i in range(16):
                va = vp.tile([128, HPC * 65], BF16, tag=f"va{i}", name=f"va{i}")
                nc.vector.memset(
                    va.rearrange("p (h c) -> p h c", c=65)[:, :, 64:65], 1.0)
                va_sb.append(va)
            # 68-col head stride keeps fp8 weight AP offsets 4B-aligned;
            # cols 64-65 are ones (M=66 even), 66-67 unused padding
            va8_sb = []
            for j in range(8):
                va8 = vp8.tile([128, 2, HPC * 68], FP8, tag=f"va8_{j}",
                               name=f"va8_{j}")
                nc.vector.memset(
                    va8.rearrange("p s (h c) -> p s h c", c=68)[:, :, :, 64:66],
                    1.0)
                va8_sb.append(va8)

            for rep in range(reps):
                # per-pr tiles; the pair's heads sit at partitions 0-63 and
                # 64-127 (dims 0-63 each; plain fp8 scores, K=64)
                qt8 = [qkp.tile([128, T], FP8, tag=f"qt8_{p}",
                                name=f"qt8_{p}") for p in range(2)]
                kt8 = [qkp.tile([128, T], FP8, tag=f"kt8_{p}",
                                name=f"kt8_{p}") for p in range(2)]
                yt = [ytp.tile([128, T], BF16, tag=f"yt{m}", name=f"yt{m}")
                      for m in range(2)]

                with ExitStack() as pstack:
                    pss = pstack.enter_context(
                        tc.tile_pool(name="pss", bufs=2, space="PSUM"))
                    pso = pstack.enter_context(
                        tc.tile_pool(name="pso", bufs=2, space="PSUM"))
                    mix = pstack.enter_context(
                        tc.tile_pool(name="mix", bufs=2, space="PSUM"))
                    pending_proj = []
                    proj_n = [0]

                    def emit_proj_group(tt, nn, osb):
                        if NO_PROJ:
                            return
                        psc = mix.tile([128, 512], F32, tag="mix")
                        for pr2 in range(2):
                            nc.tensor.matmul(
                                psc[:],
                                yt[pr2][:, tt * 128:(tt + 1) * 128],
                                wp_sb[pr2][:, nn * 512:(nn + 1) * 512],
                                start=(pr2 == 0),
                                stop=(pr2 == 1),
                            )
                        nc.vector.tensor_copy(
                            osb[:, nn * 512:(nn + 1) * 512], psc[:])
                        if nn == 1:
                            nc.sync.dma_start(
                                out=out[tt * 128:(tt + 1) * 128, :], in_=osb[:])

                    # all x loads issued up front; DMA overlaps compute.
                    # xt16 comes in as 8 full-T rows (one DMA per kc row)
                    xt8_all = []
                    xt16_rows = []
                    for kc in range(8):
                        t16 = xp16.tile([128, T], BF16, tag="xt16")
                        nc.sync.dma_start(
                            out=t16[:], in_=xT16[kc * 128:(kc + 1) * 128, :])
                        xt16_rows.append(t16)
                    for nb in range(4):
                        xt8 = []
                        for kp in range(4):
                            t8 = xp8.tile([128, 2, 512], FP8, tag="xt8")
                            nc.sync.dma_start(
                                out=t8[:],
                                in_=xT8[kp * 256:(kp + 1) * 256,
                                        nb * 512:(nb + 1) * 512].rearrange(
                                    "(two p) c -> p two c", two=2),
                            )
                            xt8.append(t8)
                        xt8_all.append(xt8)

                    for nb in range(4):
                        # ---------- phase 1 for this nb ----------
                        xt8 = xt8_all[nb]
                        xt16 = [t[:, nb * 512:(nb + 1) * 512]
                                for t in xt16_rows]

                        # QK: 2 secs x 2 pr-groups x 4 kpairs x 2 nchunks,
                        # fp8 DR; both pr-groups of a sec land in one
                        # [128,1024] pss tile (one bank each)
                        for sec, dst in ((0, qt8), (1, kt8)):
                            ps = pss.tile([128, 1024], F32, tag="pss")
                            for g in range(2):
                                for kp in range(4):
                                    for nck in range(2):
                                        # start=True zeroes the whole 2KB
                                        # PSUM bank: only the first matmul
                                        # per g-bank may set it
                                        nc.tensor.matmul(
                                            ps[:, g * 512 + nck * 256:
                                               g * 512 + (nck + 1) * 256],
                                            wa8_sb[kp][:, :,
                                                       sec * 256 + g * 128:
                                                       sec * 256 + (g + 1) * 128],
                                            xt8[kp][:, :,
                                                    nck * 256:(nck + 1) * 256],
                                            start=(kp == 0 and nck == 0),
                                            stop=(kp == 3),
                                            perf_mode=DR,
                                            skip_group_check=not (
                                                kp == 0 and nck == 0),
                                        )
                            for g in range(2):
                                nc.vector.tensor_copy(
                                    dst[g][:, nb * 512:(nb + 1) * 512],
                                    ps[:, g * 512:(g + 1) * 512])

                        # V: 4 t-tiles x 8 kc, bf16
                        for i in range(4):
                            ps = mix.tile([128, 512], F32, tag="mix")
                            for kc in range(8):
                                nc.tensor.matmul(
                                    ps[:, 0:HG],
                                    xt16[kc][:, i * 128:(i + 1) * 128],
                                    wav_sb[kc][:],
                                    start=(kc == 0),
                                    stop=(kc == 7),
                                    skip_group_check=(0 < kc < 7),
                                )
                            ti = nb * 4 + i
                            va = va_sb[ti]
                            nc.vector.tensor_copy(
                                va.rearrange("p (h c) -> p h c", c=65)[:, :, 0:64],
                                ps[:, 0:HG].rearrange(
                                    "p (h c) -> p h c", c=64)[:, :, :],
                            )
                            va8 = va8_sb[ti // 2]
                            nc.vector.tensor_copy(
                                va8.rearrange("p s (h c) -> p s h c",
                                              c=68)[:, ti % 2, :, 0:64],
                                ps[:, 0:HG].rearrange(
                                    "p (h c) -> p h c", c=64)[:, :, :],
                            )

                        # ---------- phase 2 for qb == nb ----------
                        qb = nb
                        nkc = 4 * qb + 4
                        for pr in range(2):
                            po = [pso.tile([66, 512], F32, tag="po",
                                           name=f"po{hh}") for hh in range(2)]
                            pts = {}

                            def do_scores(kc, pt8=None):
                                if NO_SCOREPV:
                                    pts[kc] = (None, 0, 0)
                                    return
                                j = kc - 4 * qb
                                full = j < 0
                                ncols = 512 if full else 512 - 128 * j
                                a0 = 0 if full else 128 * j
                                q0 = qb * 512 + a0
                                ps = pss.tile([128, 1024], F32, tag="pss")
                                for hh in range(2):
                                    nc.tensor.matmul(
                                        ps[:, hh * 512:hh * 512 + ncols],
                                        kt8[pr][hh * 64:(hh + 1) * 64,
                                                kc * 128:(kc + 1) * 128],
                                        qt8[pr][hh * 64:(hh + 1) * 64,
                                                q0:q0 + ncols],
                                        start=True,
                                        stop=True,
                                        perf_mode=SCORES_PM,
                                    )
                                if pt8 is not None:
                                    # fp8 exp for a DR full block
                                    nc.scalar.activation(
                                        pt8[:, kc % 2, :], ps[:], EXP,
                                        scale=float(SCALE))
                                    pts[kc] = (pt8, ncols, a0)
                                    return
                                pt = ptp.tile([128, 1024], BF16, tag="pt")
                                if NO_EXP:
                                    nc.scalar.activation(
                                        pt[:, 0:ncols],
                                        ps[:, 0:ncols], EXP,
                                        scale=float(SCALE))
                                    pts[kc] = (pt, ncols, a0)
                                    return
                                if full:
                                    nc.scalar.activation(pt[:], ps[:], EXP,
                                                         scale=float(SCALE))
                                else:
                                    pt3 = pt.rearrange(
                                        "p (h c) -> p h c", c=512)[:, :, 0:ncols]
                                    ps3 = ps.rearrange(
                                        "p (h c) -> p h c", c=512)[:, :, 0:ncols]
                                    nc.scalar.activation(pt3, ps3, EXP,
                                                         scale=float(SCALE))
                                    # causal mask on the diagonal 128x128
                                    # block of both heads: keep where
                                    # (query col) - (key partition) >= 0
                                    nc.gpsimd.affine_select(
                                        pt.rearrange(
                                            "p (h c) -> p h c",
                                            c=512)[:, :, 0:128],
                                        pt.rearrange(
                                            "p (h c) -> p h c",
                                            c=512)[:, :, 0:128],
                                        pattern=[[0, 2], [1, 128]],
                                        compare_op=mybir.AluOpType.is_ge,
                                        fill=0.0,
                                        base=0,
                                        channel_multiplier=-1,
                                    )
                                pts[kc] = (pt, ncols, a0)

                            def do_pv(kc, first):
                                pt, ncols, a0 = pts.pop(kc)
                                if NO_SCOREPV:
                                    return
                                for hh in range(2):
                                    h = 2 * pr + hh
                                    pcol = 0 if NO_EXP else hh * 512
                                    nc.tensor.matmul(
                                        po[hh][0:65, a0:512],
                                        va_sb[kc][:, h * 65:(h + 1) * 65],
                                        pt[:, pcol:pcol + ncols],
                                        start=first,
                                        stop=(kc == nkc - 1),
                                        skip_group_check=not first,
                                    )

                            def do_pv_pair(jp, last):
                                # fp8 DoubleRow over kc pair (2jp, 2jp+1):
                                # contraction 256 keys per column pass
                                pt8, _, _ = pts.pop(2 * jp + 1)
                                pts.pop(2 * jp, None)
                                for hh in range(2):
                                    h = 2 * pr + hh
                                    for c0 in (0, 256):
                                        nc.tensor.matmul(
                                            po[hh][0:66, c0:c0 + 256],
                                            va8_sb[jp][:, :,
                                                       h * 68:h * 68 + 66],
                                            pt8[:, :, hh * 512 + c0:
                                                hh * 512 + c0 + 256],
                                            start=(jp == 0 and c0 == 0),
                                            stop=last,
                                            perf_mode=DR,
                                            skip_group_check=not (
                                                jp == 0 and c0 == 0),
                                        )

                            use_dr = qb > 0 and not (NO_EXP or NO_SCOREPV)
                            kc = 0
                            while kc < nkc:
                                kc2 = min(kc + 2, nkc)
                                pair_full = use_dr and kc2 == kc + 2 and \
                                    (kc2 - 1) < 4 * qb
                                if pair_full:
                                    pt8 = ptp8.tile([128, 2, 1024], FP8,
                                                    tag="pt8")
                                    for k in range(kc, kc2):
                                        do_scores(k, pt8=pt8)
                                    do_pv_pair(kc // 2, last=False)
                                else:
                                    for k in range(kc, kc2):
                                        do_scores(k)
                                    for k in range(kc, kc2):
                                        do_pv(k, first=(k == 0 and not use_dr))
                                if pending_proj:
                                    emit_proj_group(*pending_proj.pop(0))
                                kc = kc2

                            # normalize: po -> sbuf (frees PSUM early), recs
                            # (DVE), broadcast via DRAM round-trip DMA (no
                            # PE), then all-SBUF fast-mode muls
                            if NO_SCOREPV:
                                continue
                            po_sb = posb.tile([128, 512], BF16, tag="posb")
                            for hh in range(2):
                                nc.vector.tensor_copy(
                                    po_sb[hh * 64:(hh + 1) * 64, :],
                                    po[hh][0:64, :])
                            rec = recp.tile([1, 1024], BF16, tag="rec")
                            with nc.allow_low_precision(
                                    reason="softmax denom recip in bf16"):
                                for hh in range(2):
                                    nc.vector.reciprocal(
                                        rec[0:1, hh * 512:(hh + 1) * 512],
                                        po[hh][64:65, :])
                            srow = scr[(qb + pr) % 2:(qb + pr) % 2 + 1, :]
                            nc.sync.dma_start(out=srow, in_=rec[:])
                            bs = bcp.tile([128, 1024], BF16, tag="bc")
                            nc.sync.dma_start(
                                out=bs[:],
                                in_=srow.squeeze(0).partition_broadcast(128))
                            for hh in range(2):
                                nc.vector.tensor_mul(
                                    yt[pr][hh * 64:(hh + 1) * 64,
                                           qb * 512:(qb + 1) * 512],
                                    po_sb[hh * 64:(hh + 1) * 64, :],
                                    bs[hh * 64:(hh + 1) * 64,
                                       hh * 512:(hh + 1) * 512],
                                )

                        for tt in range(4 * qb, 4 * qb + 4):
                            osb = outp.tile([128, L], F16, tag="osb")
                            for nn in range(2):
                                pending_proj.append((tt, nn, osb))

                    while pending_proj:
                        emit_proj_group(*pending_proj.pop(0))

    import os as _os
    if not _os.environ.get("KERNEL_SKIP_WAITFIX"):
        _fix_matmul_waits(nc)
    return nc


def _fix_matmul_waits(nc):
    """walrus caps sync-wait commands at one per hardware instruction.
    Tile can emit more. For any instruction holding >1 wait, insert
    same-engine NoOps immediately before it, each carrying one excess wait
    (the waits still all execute before the instruction dispatches).
    """
    import bass_rust
    import concourse.mybir as mybir

    SKIP = (mybir.InstEventSemaphore, mybir.InstCall,
            mybir.InstUnconditionalBranch)
    nop_id = [0]

    for f in nc.m.functions:
        for blk in f.blocks:
            insts = list(blk.instructions)
            out = []
            changed = False
            for inst in insts:
                si = inst.sync_info
                eng = getattr(inst, "engine", None)
                if si is None or eng is None or isinstance(inst, SKIP):
                    out.append(inst)
                    continue
                waits = list(si.on_wait)
                kept = waits
                if len(kept) > 1:
                    for w in kept[:-1]:
                        nop = mybir.InstNoOp(name=f"I-waitnop-{nop_id[0]}")
                        nop_id[0] += 1
                        nop.engine = eng
                        nop.sync_info = bass_rust.SyncInfo(
                            on_wait=[w], on_update=[])
                        out.append(nop)
                    kept = kept[-1:]
                if len(kept) != len(waits):
                    inst.sync_info = bass_rust.SyncInfo(
                        on_wait=kept, on_update=list(si.on_update))
                    changed = True
                out.append(inst)
            if changed or len(out) != len(insts):
                blk.instructions = out


def make_in_maps(x, W_attn, W_proj):
    x = np.ascontiguousarray(np.asarray(x, dtype=np.float32))
    W_attn = np.ascontiguousarray(np.asarray(W_attn, dtype=np.float32))
    W_proj = np.ascontiguousarray(np.asarray(W_proj, dtype=np.float32))
    import ml_dtypes
    bf16 = ml_dtypes.bfloat16
    f8 = ml_dtypes.float8_e4m3
    # [k, q] layout: valid (1.0) where q >= k, else 0 -- multiplies exp'd
    # scores after the fact.
    msk = np.triu(np.ones((128, 128), np.float32)).astype(bf16)
    in_maps = []
    for c in range(N_CORES):
        b, hg = c // 4, c % 4
        cs = slice(hg * HG, (hg + 1) * HG)
        wq = W_attn[:, 0 * L:1 * L][:, cs]      # [L, 256] this core's q cols
        wk = W_attn[:, 1 * L:2 * L][:, cs]
        wv = W_attn[:, 2 * L:3 * L][:, cs]
        # qk col layout is already [g(2) x hh(2) x d(64)] = head-major
        wa8 = np.concatenate([wq, wk], axis=1)  # [L, 512]
        in_maps.append({
            "xT8": np.ascontiguousarray(x[b].T.astype(f8)),
            "xT16": np.ascontiguousarray(x[b].T.astype(bf16)),
            "wa8": np.ascontiguousarray(wa8.astype(f8)),
            "wav": np.ascontiguousarray(wv.astype(bf16)),
            "wp": np.ascontiguousarray(W_proj[cs, :].astype(bf16)),
            "msk": np.ascontiguousarray(msk),
        })
    return in_maps


_NC_CACHE = None


def kernel(x, W_attn, W_proj, **run_kwargs):
    global _NC_CACHE
    from concourse.bass_utils import run_bass_kernel_spmd

    if _NC_CACHE is None:
        _NC_CACHE = build_nc()
    nc = _NC_CACHE
    in_maps = make_in_maps(x, W_attn, W_proj)
    res = run_bass_kernel_spmd(nc, in_maps, list(range(N_CORES)), **run_kwargs)
    results = res.results if hasattr(res, "results") else res
    out = np.zeros((B, T, L), np.float32)
    for c in range(N_CORES):
        out[c // 4] += results[c]["out"].astype(np.float32)
    if run_kwargs:
        kernel.last_results = res
    return out


# revision 64
# speedup vs baseline: 1.0123x; 1.0123x over previous
"""Causal self-attention (B=2, T=2048, L=1024, H=16) on 8 TRN2 NeuronCores.

Sharding: tensor-parallel over heads (4 heads/core) x data-parallel over batch
(cores 0-3 -> batch 0, cores 4-7 -> batch 1). Each core computes its heads'
attention plus the partial output projection; the host sums the 4 f32
partials per batch.

v4 layout (HW-measured: per-matmul overhead dominates, so the design
minimizes streamed PSUM columns and instruction count):
  - QK projection in fp8e4m3 DoubleRow (256-contraction per matmul, halves
    column passes); V projection bf16 (fp8 V fails the 2e-2 gate).
  - Scores: plain fp8, K=64, per-(pr,hh) with heads at partition bases
    {0,64}; one matmul per (kc, head), N<=512.
  - PV hybrid: q-block 0 (queries 0-511) in bf16 (small softmax
    denominators amplify fp8 noise there); full blocks of q-blocks 1-3 via
    fp8 DoubleRow over kc pairs (256-key contraction per pass, M=66 with
    a 68-col padded head stride for 4B-aligned fp8 weight APs); diagonal
    blocks bf16. ones-column rides in va -> free softmax denominator.
  - phase1 (per nb) interleaved with phase2 (per qb); all x DMAs up front.
  - exp on Act (fp8 out for DR blocks); causal mask via gpsimd
    affine_select (Pool); normalize: po->SBUF on DVE, reciprocal DVE,
    denominator broadcast via DRAM round-trip DMA (keeps PE out of the
    normalize chain), yt muls in DVE fast mode.
  - PSUM note: start=True zeroes the whole 2KB bank, so only the first
    matmul touching a bank may set it.
"""

import sys

for _p in ("/opt/trn_rl_repo",):
    if _p not in sys.path:
        sys.path.insert(0, _p)

import numpy as np

import concourse.bass as bass
import concourse.mybir as mybir
import concourse.tile as tile

F32 = mybir.dt.float32
BF16 = mybir.dt.bfloat16
FP8 = mybir.dt.float8e4
EXP = mybir.ActivationFunctionType.Exp
DR = mybir.MatmulPerfMode.DoubleRow
import os as _os0
SCORES_PM = (mybir.MatmulPerfMode.DoublePixel
             if _os0.environ.get("SCORES_DP") else None)
NO_PROJ = bool(_os0.environ.get("KERNEL_NO_PROJ"))
NO_EXP = bool(_os0.environ.get("KERNEL_NO_EXP"))
NO_SCOREPV = bool(_os0.environ.get("KERNEL_NO_SCOREPV"))

B, T, L = 2, 2048, 1024
H = 16
DH = 64                      # head dim
HPC = 4                      # heads per core
HG = HPC * DH                # 256 cols per core per q/k/v
N_CORES = 8
NQB = T // 512               # 4 q-blocks
SCALE = 1.0 / np.sqrt(np.float32(L))  # rsqrt(L) per reference


def build_nc(reps=1):
    nc = bass.Bass("TRN2", target_bir_lowering=False, debug=False)

    xT8 = nc.dram_tensor("xT8", [L, T], FP8, kind="ExternalInput").ap()
    xT16 = nc.dram_tensor("xT16", [L, T], BF16, kind="ExternalInput").ap()
    wa8 = nc.dram_tensor("wa8", [L, 2 * HG], FP8, kind="ExternalInput").ap()
    wav = nc.dram_tensor("wav", [L, HG], BF16, kind="ExternalInput").ap()
    wp = nc.dram_tensor("wp", [HG, L], BF16, kind="ExternalInput").ap()
    msk = nc.dram_tensor("msk", [128, 128], BF16, kind="ExternalInput").ap()
    F16 = mybir.dt.float16
    out = nc.dram_tensor("out", [T, L], F16, kind="ExternalOutput").ap()
    scr = nc.dram_tensor("scr", [2, 1024], BF16, kind="Internal").ap()

    from contextlib import ExitStack

    with tile.TileContext(nc) as tc:
        with ExitStack() as stack:
            consts = stack.enter_context(tc.tile_pool(name="consts", bufs=1))
            xp8 = stack.enter_context(tc.tile_pool(name="xp8", bufs=12))
            xp16 = stack.enter_context(tc.tile_pool(name="xp16", bufs=24))
            wap = stack.enter_context(tc.tile_pool(name="wap", bufs=12))
            qkp = stack.enter_context(tc.tile_pool(name="qk", bufs=4))
            vp = stack.enter_context(tc.tile_pool(name="vp", bufs=1))
            ptp = stack.enter_context(tc.tile_pool(name="ptp", bufs=4))
            ptp8 = stack.enter_context(tc.tile_pool(name="ptp8", bufs=3))
            vp8 = stack.enter_context(tc.tile_pool(name="vp8", bufs=1))
            ytp = stack.enter_context(tc.tile_pool(name="ytp", bufs=4))
            recp = stack.enter_context(tc.tile_pool(name="recp", bufs=2))
            posb = stack.enter_context(tc.tile_pool(name="posb", bufs=4))
            bcp = stack.enter_context(tc.tile_pool(name="bcp", bufs=2))
            outp = stack.enter_context(tc.tile_pool(name="outp", bufs=4))
            # ---- constants & weights (resident across reps) ----
            msk_sb = consts.tile([128, 128], BF16)
            nc.gpsimd.dma_start(out=msk_sb[:], in_=msk[:])
            on1 = consts.tile([1, 64], BF16)
            nc.vector.memset(on1[:], 1.0)

            wa8_sb = []
            for kp in range(4):
                w = wap.tile([128, 2, 512], FP8, tag="wa8")
                nc.sync.dma_start(
                    out=w[:],
                    in_=wa8[kp * 256:(kp + 1) * 256, :].rearrange(
                        "(two p) c -> p two c", two=2),
                )
                wa8_sb.append(w)
            wav_sb = []
            for kc in range(8):
                w = wap.tile([128, HG], BF16, tag="wav")
                nc.sync.dma_start(out=w[:], in_=wav[kc * 128:(kc + 1) * 128, :])
                wav_sb.append(w)
            wp_sb = []
            for i in range(2):
                w = wap.tile([128, L], BF16, tag="wp")
                nc.gpsimd.dma_start(out=w[:], in_=wp[i * 128:(i + 1) * 128, :])
                wp_sb.append(w)

            # va tiles persist across reps; ones column written once.
            # bf16 singles (diag + qb0 blocks) and fp8 kc-pairs (DR full
            # blocks, queries >= 512).
            va_sb = []
            for i in range(16):
                va = vp.tile([128, HPC * 65], BF16, tag=f"va{i}", name=f"va{i}")
                nc.vector.memset(
                    va.rearrange("p (h c) -> p h c", c=65)[:, :, 64:65], 1.0)
                va_sb.append(va)
            # 68-col head stride keeps fp8 weight AP offsets 4B-aligned;
            # cols 64-65 are ones (M=66 even), 66-67 unused padding
            va8_sb = []
            for j in range(8):
                va8 = vp8.tile([128, 2, HPC * 68], FP8, tag=f"va8_{j}",
                               name=f"va8_{j}")
                nc.vector.memset(
                    va8.rearrange("p s (h c) -> p s h c", c=68)[:, :, :, 64:66],
                    1.0)
                va8_sb.append(va8)

            for rep in range(reps):
                # per-pr tiles; the pair's heads sit at partitions 0-63 and
                # 64-127 (dims 0-63 each; plain fp8 scores, K=64)
                qt8 = [qkp.tile([128, T], FP8, tag=f"qt8_{p}",
                                name=f"qt8_{p}") for p in range(2)]
                kt8 = [qkp.tile([128, T], FP8, tag=f"kt8_{p}",
                                name=f"kt8_{p}") for p in range(2)]
                yt = [ytp.tile([128, T], BF16, tag=f"yt{m}", name=f"yt{m}")
                      for m in range(2)]

                with ExitStack() as pstack:
                    pss = pstack.enter_context(
                        tc.tile_pool(name="pss", bufs=2, space="PSUM"))
                    pso = pstack.enter_context(
                        tc.tile_pool(name="pso", bufs=2, space="PSUM"))
                    mix = pstack.enter_context(
                        tc.tile_pool(name="mix", bufs=2, space="PSUM"))
                    pending_proj = []
                    proj_n = [0]

                    def emit_proj_group(tt, nn, osb):
                        if NO_PROJ:
                            return
                        psc = mix.tile([128, 512], F32, tag="mix")
                        for pr2 in range(2):
                            nc.tensor.matmul(
                                psc[:],
                                yt[pr2][:, tt * 128:(tt + 1) * 128],
                                wp_sb[pr2][:, nn * 512:(nn + 1) * 512],
                                start=(pr2 == 0),
                                stop=(pr2 == 1),
                            )
                        nc.vector.tensor_copy(
                            osb[:, nn * 512:(nn + 1) * 512], psc[:])
                        if nn == 1:
                            nc.sync.dma_start(
                                out=out[tt * 128:(tt + 1) * 128, :], in_=osb[:])

                    # all x loads issued up front; DMA overlaps compute
                    xt8_all, xt16_all = [], []
                    for nb in range(4):
                        xt8 = []
                        for kp in range(4):
                            t8 = xp8.tile([128, 2, 512], FP8, tag="xt8")
                            nc.sync.dma_start(
                                out=t8[:],
                                in_=xT8[kp * 256:(kp + 1) * 256,
                                        nb * 512:(nb + 1) * 512].rearrange(
                                    "(two p) c -> p two c", two=2),
                            )
                            xt8.append(t8)
                        xt8_all.append(xt8)
                        xt16 = []
                        for kc in range(8):
                            t16 = xp16.tile([128, 512], BF16, tag="xt16")
                            nc.sync.dma_start(
                                out=t16[:],
                                in_=xT16[kc * 128:(kc + 1) * 128,
                                         nb * 512:(nb + 1) * 512])
                            xt16.append(t16)
                        xt16_all.append(xt16)

                    for nb in range(4):
                        # ---------- phase 1 for this nb ----------
                        xt8 = xt8_all[nb]
                        xt16 = xt16_all[nb]

                        # QK: 2 secs x 2 pr-groups x 4 kpairs x 2 nchunks,
                        # fp8 DR; both pr-groups of a sec land in one
                        # [128,1024] pss tile (one bank each)
                        for sec, dst in ((0, qt8), (1, kt8)):
                            ps = pss.tile([128, 1024], F32, tag="pss")
                            for g in range(2):
                                for kp in range(4):
                                    for nck in range(2):
                                        # start=True zeroes the whole 2KB
                                        # PSUM bank: only the first matmul
                                        # per g-bank may set it
                                        nc.tensor.matmul(
                                            ps[:, g * 512 + nck * 256:
                                               g * 512 + (nck + 1) * 256],
                                            wa8_sb[kp][:, :,
                                                       sec * 256 + g * 128:
                                                       sec * 256 + (g + 1) * 128],
                                            xt8[kp][:, :,
                                                    nck * 256:(nck + 1) * 256],
                                            start=(kp == 0 and nck == 0),
                                            stop=(kp == 3),
                                            perf_mode=DR,
                                            skip_group_check=not (
                                                kp == 0 and nck == 0),
                                        )
                            for g in range(2):
                                nc.vector.tensor_copy(
                                    dst[g][:, nb * 512:(nb + 1) * 512],
                                    ps[:, g * 512:(g + 1) * 512])

                        # V: 4 t-tiles x 8 kc, bf16
                        for i in range(4):
                            ps = mix.tile([128, 512], F32, tag="mix")
                            for kc in range(8):
                                nc.tensor.matmul(
                                    ps[:, 0:HG],
                                    xt16[kc][:, i * 128:(i + 1) * 128],
                                    wav_sb[kc][:],
                                    start=(kc == 0),
                                    stop=(kc == 7),
                                    skip_group_check=(0 < kc < 7),
                                )
                            ti = nb * 4 + i
                            va = va_sb[ti]
                            nc.vector.tensor_copy(
                                va.rearrange("p (h c) -> p h c", c=65)[:, :, 0:64],
                                ps[:, 0:HG].rearrange(
                                    "p (h c) -> p h c", c=64)[:, :, :],
                            )
                            va8 = va8_sb[ti // 2]
                            nc.vector.tensor_copy(
                                va8.rearrange("p s (h c) -> p s h c",
                                              c=68)[:, ti % 2, :, 0:64],
                                ps[:, 0:HG].rearrange(
                                    "p (h c) -> p h c", c=64)[:, :, :],
                            )

                        # ---------- phase 2 for qb == nb ----------
                        qb = nb
                        nkc = 4 * qb + 4
                        for pr in range(2):
                            po = [pso.tile([66, 512], F32, tag="po",
                                           name=f"po{hh}") for hh in range(2)]
                            pts = {}

                            def do_scores(kc, pt8=None):
                                if NO_SCOREPV:
                                    pts[kc] = (None, 0, 0)
                                    return
                                j = kc - 4 * qb
                                full = j < 0
                                ncols = 512 if full else 512 - 128 * j
                                a0 = 0 if full else 128 * j
                                q0 = qb * 512 + a0
                                ps = pss.tile([128, 1024], F32, tag="pss")
                                for hh in range(2):
                                    nc.tensor.matmul(
                                        ps[:, hh * 512:hh * 512 + ncols],
                                        kt8[pr][hh * 64:(hh + 1) * 64,
                                                kc * 128:(kc + 1) * 128],
                                        qt8[pr][hh * 64:(hh + 1) * 64,
                                                q0:q0 + ncols],
                                        start=True,
                                        stop=True,
                                        perf_mode=SCORES_PM,
                                    )
                                if pt8 is not None:
                                    # fp8 exp for a DR full block
                                    nc.scalar.activation(
                                        pt8[:, kc % 2, :], ps[:], EXP,
                                        scale=float(SCALE))
                                    pts[kc] = (pt8, ncols, a0)
                                    return
                                pt = ptp.tile([128, 1024], BF16, tag="pt")
                                if NO_EXP:
                                    nc.scalar.activation(
                                        pt[:, 0:ncols],
                                        ps[:, 0:ncols], EXP,
                                        scale=float(SCALE))
                                    pts[kc] = (pt, ncols, a0)
                                    return
                                if full:
                                    nc.scalar.activation(pt[:], ps[:], EXP,
                                                         scale=float(SCALE))
                                else:
                                    pt3 = pt.rearrange(
                                        "p (h c) -> p h c", c=512)[:, :, 0:ncols]
                                    ps3 = ps.rearrange(
                                        "p (h c) -> p h c", c=512)[:, :, 0:ncols]
                                    nc.scalar.activation(pt3, ps3, EXP,
                                                         scale=float(SCALE))
                                    # causal mask on the diagonal 128x128
                                    # block of both heads: keep where
                                    # (query col) - (key partition) >= 0
                                    nc.gpsimd.affine_select(
                                        pt.rearrange(
                                            "p (h c) -> p h c",
                                            c=512)[:, :, 0:128],
                                        pt.rearrange(
                                            "p (h c) -> p h c",
                                            c=512)[:, :, 0:128],
                                        pattern=[[0, 2], [1, 128]],
                                        compare_op=mybir.AluOpType.is_ge,
                                        fill=0.0,
                                        base=0,
                                        channel_multiplier=-1,
                                    )
                                pts[kc] = (pt, ncols, a0)

                            def do_pv(kc, first):
                                pt, ncols, a0 = pts.pop(kc)
                                if NO_SCOREPV:
                                    return
                                for hh in range(2):
                                    h = 2 * pr + hh
                                    pcol = 0 if NO_EXP else hh * 512
                                    nc.tensor.matmul(
                                        po[hh][0:65, a0:512],
                                        va_sb[kc][:, h * 65:(h + 1) * 65],
                                        pt[:, pcol:pcol + ncols],
                                        start=first,
                                        stop=(kc == nkc - 1),
                                        skip_group_check=not first,
                                    )

                            def do_pv_pair(jp, last):
                                # fp8 DoubleRow over kc pair (2jp, 2jp+1):
                                # contraction 256 keys per column pass
                                pt8, _, _ = pts.pop(2 * jp + 1)
                                pts.pop(2 * jp, None)
                                for hh in range(2):
                                    h = 2 * pr + hh
                                    for c0 in (0, 256):
                                        nc.tensor.matmul(
                                            po[hh][0:66, c0:c0 + 256],
                                            va8_sb[jp][:, :,
                                                       h * 68:h * 68 + 66],
                                            pt8[:, :, hh * 512 + c0:
                                                hh * 512 + c0 + 256],
                                            start=(jp == 0 and c0 == 0),
                                            stop=last,
                                            perf_mode=DR,
                                            skip_group_check=not (
                                                jp == 0 and c0 == 0),
                                        )

                            use_dr = qb > 0 and not (NO_EXP or NO_SCOREPV)
                            kc = 0
                            while kc < nkc:
                                kc2 = min(kc + 2, nkc)
                                pair_full = use_dr and kc2 == kc + 2 and \
                                    (kc2 - 1) < 4 * qb
                                if pair_full:
                                    pt8 = ptp8.tile([128, 2, 1024], FP8,
                                                    tag="pt8")
                                    for k in range(kc, kc2):
                                        do_scores(k, pt8=pt8)
                                    do_pv_pair(kc // 2, last=False)
                                else:
                                    for k in range(kc, kc2):
                                        do_scores(k)
                                    for k in range(kc, kc2):
                                        do_pv(k, first=(k == 0 and not use_dr))
                                if pending_proj:
                                    emit_proj_group(*pending_proj.pop(0))
                                kc = kc2

                            # normalize: po -> sbuf (frees PSUM early), recs
                            # (DVE), broadcast via DRAM round-trip DMA (no
                            # PE), then all-SBUF fast-mode muls
                            if NO_SCOREPV:
                                continue
                            po_sb = posb.tile([128, 512], BF16, tag="posb")
                            for hh in range(2):
                                nc.vector.tensor_copy(
                                    po_sb[hh * 64:(hh + 1) * 64, :],
                                    po[hh][0:64, :])
                            rec = recp.tile([1, 1024], BF16, tag="rec")
                            with nc.allow_low_precision(
                                    reason="softmax denom recip in bf16"):
                                for hh in range(2):
                                    nc.vector.reciprocal(
                                        rec[0:1, hh * 512:(hh + 1) * 512],
                                        po[hh][64:65, :])
                            srow = scr[(qb + pr) % 2:(qb + pr) % 2 + 1, :]
                            nc.sync.dma_start(out=srow, in_=rec[:])
                            bs = bcp.tile([128, 1024], BF16, tag="bc")
                            nc.sync.dma_start(
                                out=bs[:],
                                in_=srow.squeeze(0).partition_broadcast(128))
                            for hh in range(2):
                                nc.vector.tensor_mul(
                                    yt[pr][hh * 64:(hh + 1) * 64,
                                           qb * 512:(qb + 1) * 512],
                                    po_sb[hh * 64:(hh + 1) * 64, :],
                                    bs[hh * 64:(hh + 1) * 64,
                                       hh * 512:(hh + 1) * 512],
                                )

                        for tt in range(4 * qb, 4 * qb + 4):
                            osb = outp.tile([128, L], F16, tag="osb")
                            for nn in range(2):
                                pending_proj.append((tt, nn, osb))

                    while pending_proj:
                        emit_proj_group(*pending_proj.pop(0))

    import os as _os
    if not _os.environ.get("KERNEL_SKIP_WAITFIX"):
        _fix_matmul_waits(nc)
    return nc


def _fix_matmul_waits(nc):
    """walrus caps sync-wait commands at one per hardware instruction.
    Tile can emit more. For any instruction holding >1 wait, insert
    same-engine NoOps immediately before it, each carrying one excess wait
    (the waits still all execute before the instruction dispatches).
    """
    import bass_rust
    import concourse.mybir as mybir

    SKIP = (mybir.InstEventSemaphore, mybir.InstCall,
            mybir.InstUnconditionalBranch)
    nop_id = [0]

    for f in nc.m.functions:
        for blk in f.blocks:
            insts = list(blk.instructions)
            out = []
            changed = False
            for inst in insts:
                si = inst.sync_info
                eng = getattr(inst, "engine", None)
                if si is None or eng is None or isinstance(inst, SKIP):
                    out.append(inst)
                    continue
                waits = list(si.on_wait)
                kept = waits
                if len(kept) > 1:
                    for w in kept[:-1]:
                        nop = mybir.InstNoOp(name=f"I-waitnop-{nop_id[0]}")
                        nop_id[0] += 1
                        nop.engine = eng
                        nop.sync_info = bass_rust.SyncInfo(
                            on_wait=[w], on_update=[])
                        out.append(nop)
                    kept = kept[-1:]
                if len(kept) != len(waits):
                    inst.sync_info = bass_rust.SyncInfo(
                        on_wait=kept, on_update=list(si.on_update))
                    changed = True
                out.append(inst)
            if changed or len(out) != len(insts):
                blk.instructions = out


def make_in_maps(x, W_attn, W_proj):
    x = np.ascontiguousarray(np.asarray(x, dtype=np.float32))
    W_attn = np.ascontiguousarray(np.asarray(W_attn, dtype=np.float32))
    W_proj = np.ascontiguousarray(np.asarray(W_proj, dtype=np.float32))
    import ml_dtypes
    bf16 = ml_dtypes.bfloat16
    f8 = ml_dtypes.float8_e4m3
    # [k, q] layout: valid (1.0) where q >= k, else 0 -- multiplies exp'd
    # scores after the fact.
    msk = np.triu(np.ones((128, 128), np.float32)).astype(bf16)
    in_maps = []
    for c in range(N_CORES):
        b, hg = c // 4, c % 4
        cs = slice(hg * HG, (hg + 1) * HG)
        wq = W_attn[:, 0 * L:1 * L][:, cs]      # [L, 256] this core's q cols
        wk = W_attn[:, 1 * L:2 * L][:, cs]
        wv = W_attn[:, 2 * L:3 * L][:, cs]
        # qk col layout is already [g(2) x hh(2) x d(64)] = head-major
        wa8 = np.concatenate([wq, wk], axis=1)  # [L, 512]
        in_maps.append({
            "xT8": np.ascontiguousarray(x[b].T.astype(f8)),
            "xT16": np.ascontiguousarray(x[b].T.astype(bf16)),
            "wa8": np.ascontiguousarray(wa8.astype(f8)),
            "wav": np.ascontiguousarray(wv.astype(bf16)),
            "wp": np.ascontiguousarray(W_proj[cs, :].astype(bf16)),
            "msk": np.ascontiguousarray(msk),
        })
    return in_maps


_NC_CACHE = None


def kernel(x, W_attn, W_proj, **run_kwargs):
    global _NC_CACHE
    from concourse.bass_utils import run_bass_kernel_spmd

    if _NC_CACHE is None:
        _NC_CACHE = build_nc()
    nc = _NC_CACHE
    in_maps = make_in_maps(x, W_attn, W_proj)
    res = run_bass_kernel_spmd(nc, in_maps, list(range(N_CORES)), **run_kwargs)
    results = res.results if hasattr(res, "results") else res
    out = np.zeros((B, T, L), np.float32)
    for c in range(N_CORES):
        out[c // 4] += results[c]["out"].astype(np.float32)
    if run_kwargs:
        kernel.last_results = res
    return out


# revision 66
# speedup vs baseline: 1.0622x; 1.0493x over previous
"""Causal self-attention (B=2, T=2048, L=1024, H=16) on 8 TRN2 NeuronCores.

Sharding: tensor-parallel over heads (4 heads/core) x data-parallel over batch
(cores 0-3 -> batch 0, cores 4-7 -> batch 1). Each core computes its heads'
attention plus the partial output projection; the host sums the 4 f32
partials per batch.

v4 layout (HW-measured: per-matmul overhead dominates, so the design
minimizes streamed PSUM columns and instruction count):
  - QK projection in fp8e4m3 DoubleRow (256-contraction per matmul, halves
    column passes); V projection bf16 (fp8 V fails the 2e-2 gate).
  - Scores: plain fp8, K=64, per-(pr,hh) with heads at partition bases
    {0,64}; one matmul per (kc, head), N<=512.
  - PV hybrid: q-block 0 (queries 0-511) in bf16 (small softmax
    denominators amplify fp8 noise there); full blocks of q-blocks 1-3 via
    fp8 DoubleRow over kc pairs (256-key contraction per pass, M=66 with
    a 68-col padded head stride for 4B-aligned fp8 weight APs); diagonal
    blocks bf16. ones-column rides in va -> free softmax denominator.
  - phase1 (per nb) interleaved with phase2 (per qb); all x DMAs up front.
  - exp on Act (fp8 out for DR blocks); causal mask via gpsimd
    affine_select (Pool); normalize: po->SBUF on DVE, reciprocal DVE,
    denominator broadcast via DRAM round-trip DMA (keeps PE out of the
    normalize chain), yt muls in DVE fast mode.
  - PSUM note: start=True zeroes the whole 2KB bank, so only the first
    matmul touching a bank may set it.
"""

import sys

for _p in ("/opt/trn_rl_repo",):
    if _p not in sys.path:
        sys.path.insert(0, _p)

import numpy as np

import concourse.bass as bass
import concourse.mybir as mybir
import concourse.tile as tile

F32 = mybir.dt.float32
BF16 = mybir.dt.bfloat16
FP8 = mybir.dt.float8e4
EXP = mybir.ActivationFunctionType.Exp
DR = mybir.MatmulPerfMode.DoubleRow
import os as _os0
SCORES_PM = (mybir.MatmulPerfMode.DoublePixel
             if _os0.environ.get("SCORES_DP") else None)
NO_PROJ = bool(_os0.environ.get("KERNEL_NO_PROJ"))
NO_EXP = bool(_os0.environ.get("KERNEL_NO_EXP"))
NO_SCOREPV = bool(_os0.environ.get("KERNEL_NO_SCOREPV"))

B, T, L = 2, 2048, 1024
H = 16
DH = 64                      # head dim
HPC = 4                      # heads per core
HG = HPC * DH                # 256 cols per core per q/k/v
N_CORES = 8
NQB = T // 512               # 4 q-blocks
SCALE = 1.0 / np.sqrt(np.float32(L))  # rsqrt(L) per reference


def build_nc(reps=1):
    nc = bass.Bass("TRN2", target_bir_lowering=False, debug=False)

    xT8 = nc.dram_tensor("xT8", [L, T], FP8, kind="ExternalInput").ap()
    xT16 = nc.dram_tensor("xT16", [L, T], BF16, kind="ExternalInput").ap()
    wa8 = nc.dram_tensor("wa8", [L, 2 * HG], FP8, kind="ExternalInput").ap()
    wav = nc.dram_tensor("wav", [L, HG], BF16, kind="ExternalInput").ap()
    wp = nc.dram_tensor("wp", [HG, L], BF16, kind="ExternalInput").ap()
    msk = nc.dram_tensor("msk", [128, 128], BF16, kind="ExternalInput").ap()
    F16 = mybir.dt.float16
    out = nc.dram_tensor("out", [T, L], F16, kind="ExternalOutput").ap()
    scr = nc.dram_tensor("scr", [2, 1024], BF16, kind="Internal").ap()

    from contextlib import ExitStack

    with tile.TileContext(nc) as tc:
        with ExitStack() as stack:
            consts = stack.enter_context(tc.tile_pool(name="consts", bufs=1))
            xp8 = stack.enter_context(tc.tile_pool(name="xp8", bufs=12))
            xp16 = stack.enter_context(tc.tile_pool(name="xp16", bufs=24))
            wap = stack.enter_context(tc.tile_pool(name="wap", bufs=12))
            qkp = stack.enter_context(tc.tile_pool(name="qk", bufs=4))
            vp = stack.enter_context(tc.tile_pool(name="vp", bufs=1))
            ptp = stack.enter_context(tc.tile_pool(name="ptp", bufs=4))
            ptp8 = stack.enter_context(tc.tile_pool(name="ptp8", bufs=3))
            vp8 = stack.enter_context(tc.tile_pool(name="vp8", bufs=1))
            ytp = stack.enter_context(tc.tile_pool(name="ytp", bufs=4))
            recp = stack.enter_context(tc.tile_pool(name="recp", bufs=2))
            posb = stack.enter_context(tc.tile_pool(name="posb", bufs=4))
            bcp = stack.enter_context(tc.tile_pool(name="bcp", bufs=2))
            outp = stack.enter_context(tc.tile_pool(name="outp", bufs=4))
            # ---- constants & weights (resident across reps) ----
            msk_sb = consts.tile([128, 128], BF16)
            nc.gpsimd.dma_start(out=msk_sb[:], in_=msk[:])
            on1 = consts.tile([1, 64], BF16)
            nc.vector.memset(on1[:], 1.0)

            wa8_sb = []
            for kp in range(4):
                w = wap.tile([128, 2, 512], FP8, tag="wa8")
                nc.sync.dma_start(
                    out=w[:],
                    in_=wa8[kp * 256:(kp + 1) * 256, :].rearrange(
                        "(two p) c -> p two c", two=2),
                )
                wa8_sb.append(w)
            wav_sb = []
            for kc in range(8):
                w = wap.tile([128, HG], BF16, tag="wav")
                nc.sync.dma_start(out=w[:], in_=wav[kc * 128:(kc + 1) * 128, :])
                wav_sb.append(w)
            wp_sb = []
            for i in range(2):
                w = wap.tile([128, L], BF16, tag="wp")
                nc.gpsimd.dma_start(out=w[:], in_=wp[i * 128:(i + 1) * 128, :])
                wp_sb.append(w)

            # va tiles persist across reps; ones column written once.
            # bf16 singles (diag + qb0 blocks) and fp8 kc-pairs (DR full
            # blocks, queries >= 512).
            va_sb = []
            for i in range(16):
                va = vp.tile([128, HPC * 65], BF16, tag=f"va{i}", name=f"va{i}")
                nc.vector.memset(
                    va.rearrange("p (h c) -> p h c", c=65)[:, :, 64:65], 1.0)
                va_sb.append(va)
            # 68-col head stride keeps fp8 weight AP offsets 4B-aligned;
            # cols 64-65 are ones (M=66 even), 66-67 unused padding
            va8_sb = []
            for j in range(8):
                va8 = vp8.tile([128, 2, HPC * 68], FP8, tag=f"va8_{j}",
                               name=f"va8_{j}")
                nc.vector.memset(
                    va8.rearrange("p s (h c) -> p s h c", c=68)[:, :, :, 64:66],
                    1.0)
                va8_sb.append(va8)

            with ExitStack() as pstack:
                # PSUM pools and the pending-projection queue live across
                # reps: the last q-block's projection groups of rep r drain
                # during rep r+1's compute instead of an unoverlapped flush
                pss = pstack.enter_context(
                    tc.tile_pool(name="pss", bufs=2, space="PSUM"))
                pso = pstack.enter_context(
                    tc.tile_pool(name="pso", bufs=2, space="PSUM"))
                mix = pstack.enter_context(
                    tc.tile_pool(name="mix", bufs=2, space="PSUM"))
                pending_proj = []

                def emit_proj_group(tt, nn, osb, y0, y1):
                    if NO_PROJ:
                        return
                    psc = mix.tile([128, 512], F32, tag="mix")
                    for pr2, yy in ((0, y0), (1, y1)):
                        nc.tensor.matmul(
                            psc[:],
                            yy[:, tt * 128:(tt + 1) * 128],
                            wp_sb[pr2][:, nn * 512:(nn + 1) * 512],
                            start=(pr2 == 0),
                            stop=(pr2 == 1),
                        )
                    nc.vector.tensor_copy(
                        osb[:, nn * 512:(nn + 1) * 512], psc[:])
                    if nn == 1:
                        nc.sync.dma_start(
                            out=out[tt * 128:(tt + 1) * 128, :], in_=osb[:])

                for rep in range(reps):
                    # per-pr tiles; the pair's heads sit at partitions 0-63
                    # and 64-127 (dims 0-63 each; plain fp8 scores, K=64)
                    qt8 = [qkp.tile([128, T], FP8, tag=f"qt8_{p}",
                                    name=f"qt8_{p}") for p in range(2)]
                    kt8 = [qkp.tile([128, T], FP8, tag=f"kt8_{p}",
                                    name=f"kt8_{p}") for p in range(2)]
                    yt = [ytp.tile([128, T], BF16, tag=f"yt{m}",
                                   name=f"yt{m}") for m in range(2)]

                    # all x loads issued up front; DMA overlaps compute
                    xt8_all, xt16_all = [], []
                    for nb in range(4):
                        xt8 = []
                        for kp in range(4):
                            t8 = xp8.tile([128, 2, 512], FP8, tag="xt8")
                            nc.sync.dma_start(
                                out=t8[:],
                                in_=xT8[kp * 256:(kp + 1) * 256,
                                        nb * 512:(nb + 1) * 512].rearrange(
                                    "(two p) c -> p two c", two=2),
                            )
                            xt8.append(t8)
                        xt8_all.append(xt8)
                        xt16 = []
                        for kc in range(8):
                            t16 = xp16.tile([128, 512], BF16, tag="xt16")
                            nc.sync.dma_start(
                                out=t16[:],
                                in_=xT16[kc * 128:(kc + 1) * 128,
                                         nb * 512:(nb + 1) * 512])
                            xt16.append(t16)
                        xt16_all.append(xt16)

                    for nb in range(4):
                        # ---------- phase 1 for this nb ----------
                        xt8 = xt8_all[nb]
                        xt16 = xt16_all[nb]

                        # QK: 2 secs x 2 pr-groups x 4 kpairs x 2 nchunks,
                        # fp8 DR; both pr-groups of a sec land in one
                        # [128,1024] pss tile (one bank each)
                        for sec, dst in ((0, qt8), (1, kt8)):
                            ps = pss.tile([128, 1024], F32, tag="pss")
                            for g in range(2):
                                for kp in range(4):
                                    for nck in range(2):
                                        # start=True zeroes the whole 2KB
                                        # PSUM bank: only the first matmul
                                        # per g-bank may set it
                                        nc.tensor.matmul(
                                            ps[:, g * 512 + nck * 256:
                                               g * 512 + (nck + 1) * 256],
                                            wa8_sb[kp][:, :,
                                                       sec * 256 + g * 128:
                                                       sec * 256 + (g + 1) * 128],
                                            xt8[kp][:, :,
                                                    nck * 256:(nck + 1) * 256],
                                            start=(kp == 0 and nck == 0),
                                            stop=(kp == 3),
                                            perf_mode=DR,
                                            skip_group_check=not (
                                                kp == 0 and nck == 0),
                                        )
                            for g in range(2):
                                nc.vector.tensor_copy(
                                    dst[g][:, nb * 512:(nb + 1) * 512],
                                    ps[:, g * 512:(g + 1) * 512])

                        # V: 4 t-tiles x 8 kc, bf16
                        for i in range(4):
                            ps = mix.tile([128, 512], F32, tag="mix")
                            for kc in range(8):
                                nc.tensor.matmul(
                                    ps[:, 0:HG],
                                    xt16[kc][:, i * 128:(i + 1) * 128],
                                    wav_sb[kc][:],
                                    start=(kc == 0),
                                    stop=(kc == 7),
                                    skip_group_check=(0 < kc < 7),
                                )
                            ti = nb * 4 + i
                            va = va_sb[ti]
                            nc.vector.tensor_copy(
                                va.rearrange("p (h c) -> p h c", c=65)[:, :, 0:64],
                                ps[:, 0:HG].rearrange(
                                    "p (h c) -> p h c", c=64)[:, :, :],
                            )
                            va8 = va8_sb[ti // 2]
                            nc.vector.tensor_copy(
                                va8.rearrange("p s (h c) -> p s h c",
                                              c=68)[:, ti % 2, :, 0:64],
                                ps[:, 0:HG].rearrange(
                                    "p (h c) -> p h c", c=64)[:, :, :],
                            )

                        # ---------- phase 2 for qb == nb ----------
                        qb = nb
                        nkc = 4 * qb + 4
                        for pr in range(2):
                            po = [pso.tile([66, 512], F32, tag="po",
                                           name=f"po{hh}") for hh in range(2)]
                            pts = {}

                            def do_scores(kc, pt8=None):
                                if NO_SCOREPV:
                                    pts[kc] = (None, 0, 0)
                                    return
                                j = kc - 4 * qb
                                full = j < 0
                                ncols = 512 if full else 512 - 128 * j
                                a0 = 0 if full else 128 * j
                                q0 = qb * 512 + a0
                                ps = pss.tile([128, 1024], F32, tag="pss")
                                for hh in range(2):
                                    nc.tensor.matmul(
                                        ps[:, hh * 512:hh * 512 + ncols],
                                        kt8[pr][hh * 64:(hh + 1) * 64,
                                                kc * 128:(kc + 1) * 128],
                                        qt8[pr][hh * 64:(hh + 1) * 64,
                                                q0:q0 + ncols],
                                        start=True,
                                        stop=True,
                                        perf_mode=SCORES_PM,
                                    )
                                if pt8 is not None:
                                    # fp8 exp for a DR full block
                                    nc.scalar.activation(
                                        pt8[:, kc % 2, :], ps[:], EXP,
                                        scale=float(SCALE))
                                    pts[kc] = (pt8, ncols, a0)
                                    return
                                pt = ptp.tile([128, 1024], BF16, tag="pt")
                                if NO_EXP:
                                    nc.scalar.activation(
                                        pt[:, 0:ncols],
                                        ps[:, 0:ncols], EXP,
                                        scale=float(SCALE))
                                    pts[kc] = (pt, ncols, a0)
                                    return
                                if full:
                                    nc.scalar.activation(pt[:], ps[:], EXP,
                                                         scale=float(SCALE))
                                else:
                                    pt3 = pt.rearrange(
                                        "p (h c) -> p h c", c=512)[:, :, 0:ncols]
                                    ps3 = ps.rearrange(
                                        "p (h c) -> p h c", c=512)[:, :, 0:ncols]
                                    nc.scalar.activation(pt3, ps3, EXP,
                                                         scale=float(SCALE))
                                    # causal mask on the diagonal 128x128
                                    # block of both heads: keep where
                                    # (query col) - (key partition) >= 0
                                    nc.gpsimd.affine_select(
                                        pt.rearrange(
                                            "p (h c) -> p h c",
                                            c=512)[:, :, 0:128],
                                        pt.rearrange(
                                            "p (h c) -> p h c",
                                            c=512)[:, :, 0:128],
                                        pattern=[[0, 2], [1, 128]],
                                        compare_op=mybir.AluOpType.is_ge,
                                        fill=0.0,
                                        base=0,
                                        channel_multiplier=-1,
                                    )
                                pts[kc] = (pt, ncols, a0)

                            def do_pv(kc, first):
                                pt, ncols, a0 = pts.pop(kc)
                                if NO_SCOREPV:
                                    return
                                for hh in range(2):
                                    h = 2 * pr + hh
                                    pcol = 0 if NO_EXP else hh * 512
                                    nc.tensor.matmul(
                                        po[hh][0:65, a0:512],
                                        va_sb[kc][:, h * 65:(h + 1) * 65],
                                        pt[:, pcol:pcol + ncols],
                                        start=first,
                                        stop=(kc == nkc - 1),
                                        skip_group_check=not first,
                                    )

                            def do_pv_pair(jp, last):
                                # fp8 DoubleRow over kc pair (2jp, 2jp+1):
                                # contraction 256 keys per column pass
                                pt8, _, _ = pts.pop(2 * jp + 1)
                                pts.pop(2 * jp, None)
                                for hh in range(2):
                                    h = 2 * pr + hh
                                    for c0 in (0, 256):
                                        nc.tensor.matmul(
                                            po[hh][0:66, c0:c0 + 256],
                                            va8_sb[jp][:, :,
                                                       h * 68:h * 68 + 66],
                                            pt8[:, :, hh * 512 + c0:
                                                hh * 512 + c0 + 256],
                                            start=(jp == 0 and c0 == 0),
                                            stop=last,
                                            perf_mode=DR,
                                            skip_group_check=not (
                                                jp == 0 and c0 == 0),
                                        )

                            use_dr = qb > 0 and not (NO_EXP or NO_SCOREPV)
                            kc = 0
                            while kc < nkc:
                                kc2 = min(kc + 2, nkc)
                                pair_full = use_dr and kc2 == kc + 2 and \
                                    (kc2 - 1) < 4 * qb
                                if pair_full:
                                    pt8 = ptp8.tile([128, 2, 1024], FP8,
                                                    tag="pt8")
                                    for k in range(kc, kc2):
                                        do_scores(k, pt8=pt8)
                                    do_pv_pair(kc // 2, last=False)
                                else:
                                    for k in range(kc, kc2):
                                        do_scores(k)
                                    for k in range(kc, kc2):
                                        do_pv(k, first=(k == 0 and not use_dr))
                                if pending_proj:
                                    emit_proj_group(*pending_proj.pop(0))
                                kc = kc2

                            # normalize: po -> sbuf (frees PSUM early), recs
                            # (DVE), broadcast via DRAM round-trip DMA (no
                            # PE), then all-SBUF fast-mode muls
                            if NO_SCOREPV:
                                continue
                            po_sb = posb.tile([128, 512], BF16, tag="posb")
                            for hh in range(2):
                                nc.vector.tensor_copy(
                                    po_sb[hh * 64:(hh + 1) * 64, :],
                                    po[hh][0:64, :])
                            rec = recp.tile([1, 1024], BF16, tag="rec")
                            with nc.allow_low_precision(
                                    reason="softmax denom recip in bf16"):
                                for hh in range(2):
                                    nc.vector.reciprocal(
                                        rec[0:1, hh * 512:(hh + 1) * 512],
                                        po[hh][64:65, :])
                            srow = scr[(qb + pr) % 2:(qb + pr) % 2 + 1, :]
                            nc.sync.dma_start(out=srow, in_=rec[:])
                            bs = bcp.tile([128, 1024], BF16, tag="bc")
                            nc.sync.dma_start(
                                out=bs[:],
                                in_=srow.squeeze(0).partition_broadcast(128))
                            for hh in range(2):
                                nc.vector.tensor_mul(
                                    yt[pr][hh * 64:(hh + 1) * 64,
                                           qb * 512:(qb + 1) * 512],
                                    po_sb[hh * 64:(hh + 1) * 64, :],
                                    bs[hh * 64:(hh + 1) * 64,
                                       hh * 512:(hh + 1) * 512],
                                )

                        for tt in range(4 * qb, 4 * qb + 4):
                            osb = outp.tile([128, L], F16, tag="osb")
                            for nn in range(2):
                                pending_proj.append(
                                    (tt, nn, osb, yt[0], yt[1]))

                while pending_proj:
                    emit_proj_group(*pending_proj.pop(0))

    import os as _os
    if not _os.environ.get("KERNEL_SKIP_WAITFIX"):
        _fix_matmul_waits(nc)
    return nc


def _fix_matmul_waits(nc):
    """walrus caps sync-wait commands at one per hardware instruction.
    Tile can emit more. For any instruction holding >1 wait, insert
    same-engine NoOps immediately before it, each carrying one excess wait
    (the waits still all execute before the instruction dispatches).
    """
    import bass_rust
    import concourse.mybir as mybir

    SKIP = (mybir.InstEventSemaphore, mybir.InstCall,
            mybir.InstUnconditionalBranch)
    nop_id = [0]

    for f in nc.m.functions:
        for blk in f.blocks:
            insts = list(blk.instructions)
            out = []
            changed = False
            for inst in insts:
                si = inst.sync_info
                eng = getattr(inst, "engine", None)
                if si is None or eng is None or isinstance(inst, SKIP):
                    out.append(inst)
                    continue
                waits = list(si.on_wait)
                kept = waits
                if len(kept) > 1:
                    for w in kept[:-1]:
                        nop = mybir.InstNoOp(name=f"I-waitnop-{nop_id[0]}")
                        nop_id[0] += 1
                        nop.engine = eng
                        nop.sync_info = bass_rust.SyncInfo(
                            on_wait=[w], on_update=[])
                        out.append(nop)
                    kept = kept[-1:]
                if len(kept) != len(waits):
                    inst.sync_info = bass_rust.SyncInfo(
                        on_wait=kept, on_update=list(si.on_update))
                    changed = True
                out.append(inst)
            if changed or len(out) != len(insts):
                blk.instructions = out


def make_in_maps(x, W_attn, W_proj):
    x = np.ascontiguousarray(np.asarray(x, dtype=np.float32))
    W_attn = np.ascontiguousarray(np.asarray(W_attn, dtype=np.float32))
    W_proj = np.ascontiguousarray(np.asarray(W_proj, dtype=np.float32))
    import ml_dtypes
    bf16 = ml_dtypes.bfloat16
    f8 = ml_dtypes.float8_e4m3
    # [k, q] layout: valid (1.0) where q >= k, else 0 -- multiplies exp'd
    # scores after the fact.
    msk = np.triu(np.ones((128, 128), np.float32)).astype(bf16)
    in_maps = []
    for c in range(N_CORES):
        b, hg = c // 4, c % 4
        cs = slice(hg * HG, (hg + 1) * HG)
        wq = W_attn[:, 0 * L:1 * L][:, cs]      # [L, 256] this core's q cols
        wk = W_attn[:, 1 * L:2 * L][:, cs]
        wv = W_attn[:, 2 * L:3 * L][:, cs]
        # qk col layout is already [g(2) x hh(2) x d(64)] = head-major
        wa8 = np.concatenate([wq, wk], axis=1)  # [L, 512]
        in_maps.append({
            "xT8": np.ascontiguousarray(x[b].T.astype(f8)),
            "xT16": np.ascontiguousarray(x[b].T.astype(bf16)),
            "wa8": np.ascontiguousarray(wa8.astype(f8)),
            "wav": np.ascontiguousarray(wv.astype(bf16)),
            "wp": np.ascontiguousarray(W_proj[cs, :].astype(bf16)),
            "msk": np.ascontiguousarray(msk),
        })
    return in_maps


_NC_CACHE = None


def kernel(x, W_attn, W_proj, **run_kwargs):
    global _NC_CACHE
    from concourse.bass_utils import run_bass_kernel_spmd

    if _NC_CACHE is None:
        _NC_CACHE = build_nc()
    nc = _NC_CACHE
    in_maps = make_in_maps(x, W_attn, W_proj)
    res = run_bass_kernel_spmd(nc, in_maps, list(range(N_CORES)), **run_kwargs)
    results = res.results if hasattr(res, "results") else res
    out = np.zeros((B, T, L), np.float32)
    for c in range(N_CORES):
        out[c // 4] += results[c]["out"].astype(np.float32)
    if run_kwargs:
        kernel.last_results = res
    return out
